# revision 20
# baseline (speedup 1.0000x reference)
"""Trainium2 Bass kernel for nn_Decoder (4-layer dense transformer decoder).

Sharding: token-parallel over 8 cores, batch-interleaved. Core c owns token
range [QB*c, QB*(c+1)) of BOTH batches (QB = S/8), so the per-layer K/V
AllGathers run over all 8 ranks and every core consumes the gathered buffer
at identical addresses (SPMD-uniform); causality and batch separation are
pure per-core mask data. Attention is computed transposed (S^T[t,s]) so the
full-row softmax denominator (softmax-then-tril, per the reference) falls out
of ones-column matmuls. K/V gathers are split in half (head groups) so the
collective latency pipelines against projection+score compute; masks are
batch-deduplicated; PSUM->SBUF drains run on the scalar engine to keep DVE
off the critical path; vocab weights prefetch during the last layer. Final
vocab projection is last-token-only and vocab-sharded across the 8 cores.

Perf notes from trace analysis (for future iterations): the attention phase
is matmul *instruction-count* bound (~131ns/instruction floor at N=256, PE
back-to-back), not FLOP bound — fp8 DoubleRow QK (tried) halves stream time
but not instruction count and measured slower; zpv/zps DoubleRow chunk-pairs
are ISA-illegal (dual-fp8 LDWEIGHTS caps contraction at 128: 128part x 2
violates s3_lw_dual_fp8_restrictions). Merging zpv+zps into one PSUM bank
(tried twice) serializes PE ~60% — keep them in separate banks. Remaining
headroom: ~45us/layer of exposed collective latency at layer boundaries
and the 32 ones-column denominator matmuls per head. Batch-split pipelining
of the FFN->proj->gather tail (tried, 2177us vs 1928us here) LOST ~250us:
halving projection N to 256 doubles projection instruction count at the
~131ns/instruction floor, and the SBUF cost of the doubled weight pools
forced pu/pm pools down to 4, throttling the attention pipeline. A future
attempt must keep projections at N=512 (pipeline at the gather level only)
and must not shrink pu/pm below 5. LayerNorm now computes var = E[x^2]-mu^2
with Sum(x^2) on the scalar engine (Square+accum_out) and a single fused
normalize pass on DVE (-30us vs the 5-pass DVE version); the FFN ReLU also
runs on ACT. Final trace (1.891ms, PE 70.7% busy): the DVE tail
spikes are fixed (DVE <=49% everywhere; attention buckets run PE 99%/ACT 66%).
Remaining losses, measured: 50us layer-0 cold start (gathers with nothing to
overlap), ~253us of 5-20us gaps in the per-layer collective windows (CC
67-83% while PE dips -- needs gather-level pipelining keeping N=512), ~110us
of sub-us instruction-floor stalls, and the 32 ones-column denominator
matmuls per head inside the PE-saturated attention phase.
"""

import numpy as np
import ml_dtypes
from dataclasses import dataclass

BF16 = ml_dtypes.bfloat16
TRACE = False
_CACHE = {}


@dataclass(frozen=True)
class Cfg:
    L: int = 4
    S: int = 2048       # seq len per batch
    D: int = 1024
    H: int = 16
    V: int = 32000
    DH: int = 64
    NCORES: int = 8
    CGE: int = 4        # key chunks per exp/mask op
    SV_D: int = 2       # score-phase lookahead depth (heads)

    @property
    def QB(self):  # tokens per batch per core
        return self.S // self.NCORES

    @property
    def OT(self):  # tokens per core (both batches)
        return 2 * self.QB

    @property
    def CK(self):  # key-chunk size (contraction tile of attention)
        return min(128, self.QB)

    @property
    def NT2(self):  # total key chunks (both batches)
        return 2 * self.S // self.CK

    @property
    def ST(self):  # 128-row token tiles per core
        return self.OT // 128

    @property
    def DT(self):
        return self.D // 128

    @property
    def VS(self):
        return self.V // self.NCORES


FULL = Cfg()


def _vchunks(vs):
    out, v0 = [], 0
    while v0 < vs:
        vn = min(512, vs - v0)
        out.append((v0, vn))
        v0 += vn
    return out


def build_nc(cfg: Cfg):
    import concourse.bass as bass  # noqa: F401
    from concourse import bacc, mybir, tile

    f32 = mybir.dt.float32
    bf16 = mybir.dt.bfloat16
    f8 = mybir.dt.float8e4
    DR = mybir.MatmulPerfMode.DoubleRow
    L, D, H = cfg.L, cfg.D, cfg.H
    OT, QB, CK, NT2, ST, DT = cfg.OT, cfg.QB, cfg.CK, cfg.NT2, cfg.ST, cfg.DT
    VS, CGE = cfg.VS, cfg.CGE
    NC = cfg.NCORES
    DH2 = DT // 2          # weight m-tiles per gather half
    HD2 = D // 2           # V columns per gather half
    dcols = [(i * 512, min(512, D - i * 512)) for i in range((D + 511) // 512)]
    g8 = [list(range(NC))]
    cpb = QB // CK  # key chunks per rank per batch
    NJG = NT2 // CGE          # jg groups per head (both batches)
    NJGB = NJG // 2           # jg groups per batch

    def bat(j):
        return (j % (2 * cpb)) // cpb

    order = ([j for j in range(NT2) if bat(j) == 0]
             + [j for j in range(NT2) if bat(j) == 1])

    nc = bacc.Bacc("TRN2", target_bir_lowering=False, debug=False,
                   num_devices=NC)

    t_x0 = nc.dram_tensor("x0", [OT, D], f32, kind="ExternalInput")
    t_x0T = nc.dram_tensor("x0T", [D, OT], bf16, kind="ExternalInput")
    t_wq = nc.dram_tensor("wq", [L, D, D], bf16, kind="ExternalInput")
    t_wk = nc.dram_tensor("wk", [L, D, D], bf16, kind="ExternalInput")
    t_wv = nc.dram_tensor("wv", [L, D, D], bf16, kind="ExternalInput")
    t_wf = nc.dram_tensor("wf", [L, D, D], bf16, kind="ExternalInput")
    t_bq = nc.dram_tensor("bq", [L, D], f32, kind="ExternalInput")
    t_bk = nc.dram_tensor("bk", [L, D], f32, kind="ExternalInput")
    t_bv = nc.dram_tensor("bv", [1, L * D], bf16, kind="ExternalInput")
    t_bf = nc.dram_tensor("bfn", [1, L * D], bf16, kind="ExternalInput")
    t_masks = nc.dram_tensor("masks", [NT2 // 2, CK, QB], bf16,
                             kind="ExternalInput")
    t_idb = nc.dram_tensor("idenb", [128, 128], bf16, kind="ExternalInput")
    t_idf = nc.dram_tensor("idenf", [128, 128], f32, kind="ExternalInput")
    t_wout = nc.dram_tensor("wout", [D, VS], bf16, kind="ExternalInput")
    t_bout = nc.dram_tensor("bout", [1, VS], bf16, kind="ExternalInput")
    t_probs = nc.dram_tensor("probs", [2, VS], f32, kind="ExternalOutput")

    Exp = mybir.ActivationFunctionType.Exp
    Copy = mybir.ActivationFunctionType.Copy
    Square = mybir.ActivationFunctionType.Square
    Relu = mybir.ActivationFunctionType.Relu
    AX = mybir.AxisListType.X
    mult = mybir.AluOpType.mult

    with tile.TileContext(nc) as tc:
        with (
            tc.tile_pool(name="persist", bufs=1) as pp,
            tc.tile_pool(name="wpool", bufs=16) as wp,
            tc.tile_pool(name="wout_p", bufs=12) as wop,
            tc.tile_pool(name="khp", bufs=2) as khp,
            tc.tile_pool(name="vhp", bufs=3) as vhp,
            tc.tile_pool(name="pu", bufs=6) as pup,
            tc.tile_pool(name="pm", bufs=6) as pmp,
            tc.tile_pool(name="zsb", bufs=4) as zsbp,
            tc.tile_pool(name="small", bufs=4) as smp,
            tc.tile_pool(name="sbx", bufs=2) as sbx,
            tc.tile_pool(name="ps_sc", bufs=2, space="PSUM") as ps_sc,
            tc.tile_pool(name="ps_zv", bufs=1, space="PSUM") as ps_zv,
            tc.tile_pool(name="ps_zs", bufs=1, space="PSUM") as ps_zs,
            tc.tile_pool(name="ps_sm", bufs=2, space="PSUM") as ps_sm,
            tc.tile_pool(name="dram", bufs=2, space="DRAM") as dr,
        ):
            X = pp.tile([128, ST * D], f32, tag="X")
            Zacc = pp.tile([128, ST * D], bf16, tag="Zacc")
            XT = pp.tile([128, DT * OT], bf16, tag="XT")
            QT = pp.tile([128, DT * OT], bf16, tag="QT")
            MS = pp.tile([CK, (NT2 // 2) * QB], bf16, tag="MS")
            BQ = pp.tile([128, L * DT], f32, tag="BQ")
            BK = pp.tile([128, L * DT], f32, tag="BK")
            BV = pp.tile([1, L * D], bf16, tag="BV")
            BF = pp.tile([1, L * D], bf16, tag="BF")
            IDB = pp.tile([128, 128], bf16, tag="IDB")
            IDF = pp.tile([128, 128], f32, tag="IDF")
            ONE_COL = pp.tile([128, 1], bf16, tag="ONE_COL")
            ONE_ROW = pp.tile([1, 128], bf16, tag="ONE_ROW")

            # XT first, split per k-tile, so the layer-0 K projection's
            # k-loop starts as soon as its first 128-row slice lands.
            x0T_r = t_x0T.ap().rearrange("(dt p) c -> p dt c", p=128)
            for dt in range(DT):
                nc.sync.dma_start(XT[:, dt * OT:(dt + 1) * OT], x0T_r[:, dt])
            nc.sync.dma_start(
                X[:].rearrange("p (st d) -> p st d", d=D),
                t_x0.ap().rearrange("(st p) d -> p st d", p=128))
            nc.sync.dma_start(
                MS[:].rearrange("p (j c) -> p j c", c=QB),
                t_masks.ap().rearrange("j p c -> p j c"))
            nc.sync.dma_start(
                BQ[:].rearrange("p (l dt) -> p l dt", dt=DT),
                t_bq.ap().rearrange("l (dt p) -> p l dt", p=128))
            nc.sync.dma_start(
                BK[:].rearrange("p (l dt) -> p l dt", dt=DT),
                t_bk.ap().rearrange("l (dt p) -> p l dt", p=128))
            nc.sync.dma_start(BV[:], t_bv.ap())
            nc.sync.dma_start(BF[:], t_bf.ap())
            nc.sync.dma_start(IDB[:], t_idb.ap())
            nc.sync.dma_start(IDF[:], t_idf.ap())
            nc.vector.memset(ONE_COL[:], 1.0)
            nc.vector.memset(ONE_ROW[:], 1.0)

            def res_ln_cast(lbl, radd):
                # Per token-tile: residual add -> layer_norm -> bf16 cast ->
                # transpose into XT. Fused per-st so PE transpose work for
                # st=0 starts while st=1 is still in its DVE/ACT chain.
                # var = E[x^2] - mu^2 (no eps, divides by var per reference).
                # Sum(x^2) rides the scalar engine (Square + accum_out), the
                # mean stays on DVE, normalize is one fused DVE pass:
                # out = x*rv + (-mu*rv), rv = 1/var.
                for st in range(ST):
                    xs = X[:, st * D:(st + 1) * D]
                    radd(st, xs)
                    mu = smp.tile([128, 1], f32, tag="mu", name=f"mu{lbl}_{st}")
                    nc.vector.reduce_sum(mu[:], xs, axis=AX)
                    nc.vector.tensor_scalar_mul(mu[:], mu[:], 1.0 / D)
                    sq = sbx.tile([128, D], bf16, tag="sq", name=f"sq{lbl}_{st}")
                    vv = smp.tile([128, 1], f32, tag="vv", name=f"vv{lbl}_{st}")
                    nc.scalar.activation(sq[:], xs, Square,
                                         accum_out=vv[:])
                    m2 = smp.tile([128, 1], f32, tag="m2", name=f"m2{lbl}_{st}")
                    nc.vector.tensor_mul(m2[:], mu[:], mu[:])
                    nc.vector.tensor_scalar_mul(vv[:], vv[:], 1.0 / D)
                    nc.vector.tensor_sub(vv[:], vv[:], m2[:])
                    nc.vector.reciprocal(vv[:], vv[:])
                    nmr = smp.tile([128, 1], f32, tag="nmr",
                                   name=f"nmr{lbl}_{st}")
                    nc.vector.tensor_mul(nmr[:], mu[:], vv[:])
                    nc.vector.tensor_scalar_mul(nmr[:], nmr[:], -1.0)
                    nc.vector.tensor_scalar(
                        out=xs, in0=xs, scalar1=vv[:], scalar2=nmr[:],
                        op0=mult, op1=mybir.AluOpType.add)
                    cx = sbx.tile([128, D], bf16, tag="cx", name=f"cx{lbl}_{st}")
                    nc.scalar.activation(cx[:], xs, Copy)
                    for dt in range(DT):
                        tp = ps_sm.tile([128, 128], bf16, tag="sm",
                                        name=f"tpx{lbl}_{st}_{dt}")
                        nc.tensor.transpose(tp[:], cx[:, dt * 128:(dt + 1) * 128],
                                            IDB[:])
                        nc.scalar.activation(
                            XT[:, dt * OT + st * 128: dt * OT + (st + 1) * 128],
                            tp[:], Copy)

            vch = _vchunks(VS)
            wt_all = {}

            def vocab_wt(vc, dt):
                v0, vn = vch[vc]
                if (vc, dt) not in wt_all:
                    wt = wop.tile([128, 512], bf16, tag="wo",
                                  name=f"wo{vc}_{dt}")
                    nc.sync.dma_start(wt[:, :vn],
                                      t_wout.ap()[dt * 128:(dt + 1) * 128,
                                                  v0:v0 + vn])
                    wt_all[(vc, dt)] = wt
                return wt_all[(vc, dt)]

            for l in range(L):
                # ---- K projection (feature-major), gathered in 2 halves ----
                kin = [dr.tile([DH2 * 128 * OT], bf16, tag=f"kin{g}",
                               name=f"kin{l}_{g}") for g in range(2)]
                kout = [dr.tile([NC * DH2 * 128 * OT], bf16, tag=f"kout{g}",
                                name=f"kout{l}_{g}", addr_space="Shared")
                        for g in range(2)]
                vin = [dr.tile([OT * HD2], bf16, tag=f"vin{g}",
                               name=f"vin{l}_{g}") for g in range(2)]
                vout = [dr.tile([NC * OT * HD2], bf16, tag=f"vout{g}",
                                name=f"vout{l}_{g}", addr_space="Shared")
                        for g in range(2)]
                # K and V weights live simultaneously (wp bufs=16) so the
                # gather queue can run K0, V0, K1, V1: attention head 0 only
                # blocks on K0+V0, and K1/V1 hide under heads 0-7. Q proj
                # slots between the half-0 and half-1 work to keep PE fed
                # while the half-0 gathers run on CC.
                wk_t = [wp.tile([128, D], bf16, tag="w", name=f"wk{l}_{i}")
                        for i in range(DT)]
                for k in range(DT):
                    nc.sync.dma_start(
                        wk_t[k][:],
                        t_wk.ap()[l].rearrange("(k p) m -> p k m", p=128)[:, k])
                wv_t = [wp.tile([128, D], bf16, tag="w", name=f"wv{l}_{i}")
                        for i in range(DT)]
                for k in range(DT):
                    nc.sync.dma_start(
                        wv_t[k][:],
                        t_wv.ap()[l].rearrange("(k p) m -> p k m", p=128)[:, k])

                def k_half(g):
                    for mi in range(DH2):
                        m = g * DH2 + mi
                        ps = ps_sm.tile([128, OT], f32, tag="sm",
                                        name=f"kp{l}_{m}")
                        for k in range(DT):
                            nc.tensor.matmul(
                                ps[:], wk_t[k][:, m * 128:(m + 1) * 128],
                                XT[:, k * OT:(k + 1) * OT],
                                start=(k == 0), stop=(k == DT - 1))
                        ks = sbx.tile([128, OT], bf16, tag="ks",
                                      name=f"ks{l}_{m}")
                        nc.vector.tensor_scalar_add(
                            ks[:], ps[:], BK[:, l * DT + m: l * DT + m + 1])
                        nc.sync.dma_start(
                            kin[g][mi * 128 * OT:(mi + 1) * 128 * OT]
                            .rearrange("(p c) -> p c", c=OT), ks[:])
                    nc.gpsimd.collective_compute(
                        "AllGather", mybir.AluOpType.bypass, replica_groups=g8,
                        ins=[kin[g][:].opt()], outs=[kout[g][:].opt()])

                def v_half(g):
                    n0, nn = dcols[g]
                    for st in range(ST):
                        ps = ps_sm.tile([128, 512], f32, tag="sm",
                                        name=f"vp{l}_{st}_{n0}")
                        for k in range(DT):
                            nc.tensor.matmul(
                                ps[:, :nn],
                                XT[:, k * OT + st * 128: k * OT + (st + 1) * 128],
                                wv_t[k][:, n0:n0 + nn],
                                start=(k == 0), stop=False)
                        nc.tensor.matmul(ps[:, :nn], ONE_ROW[:],
                                         BV[:, l * D + n0: l * D + n0 + nn],
                                         start=False, stop=True)
                        vs = sbx.tile([128, 512], bf16, tag="vs",
                                      name=f"vsb{l}_{st}_{n0}")
                        nc.vector.tensor_copy(vs[:, :nn], ps[:, :nn])
                        nc.sync.dma_start(
                            vin[g][st * 128 * HD2:(st + 1) * 128 * HD2]
                            .rearrange("(p d) -> p d", d=HD2), vs[:, :nn])
                    nc.gpsimd.collective_compute(
                        "AllGather", mybir.AluOpType.bypass, replica_groups=g8,
                        ins=[vin[g][:].opt()], outs=[vout[g][:].opt()])

                k_half(0)
                v_half(0)
                k_half(1)
                v_half(1)

                # ---- Q projection (feature-major, stays in SBUF) ----
                # Last: wq reuses wk's tag-"w" buffers, legal only once the
                # K half-1 matmuls (wk's last readers) have been issued.
                wq_t = [wp.tile([128, D], bf16, tag="w", name=f"wq{l}_{i}")
                        for i in range(DT)]
                for k in range(DT):
                    nc.sync.dma_start(
                        wq_t[k][:],
                        t_wq.ap()[l].rearrange("(k p) m -> p k m", p=128)[:, k])
                for m in range(DT):
                    ps = ps_sm.tile([128, OT], f32, tag="sm", name=f"qp{l}_{m}")
                    for k in range(DT):
                        nc.tensor.matmul(ps[:], wq_t[k][:, m * 128:(m + 1) * 128],
                                         XT[:, k * OT:(k + 1) * OT],
                                         start=(k == 0), stop=(k == DT - 1))
                    nc.vector.tensor_scalar_add(QT[:, m * OT:(m + 1) * OT], ps[:],
                                                BQ[:, l * DT + m: l * DT + m + 1])

                if l == L - 1:
                    # Prefetch the first 12 vocab weight tiles (= the free
                    # wop bufs — no buf-wait, so the DMA queues never block)
                    # so the vocab tail starts compute immediately.
                    for i in range(12):
                        vocab_wt(i // DT, i % DT)

                # ---- attention (interleaved per-jg, as scheduled by Tile) ----
                zs_list = []
                kh = None
                for h in range(H):
                    hp, off = h // 2, (h % 2) * 64
                    if h % 2 == 0:
                        kh = khp.tile([128, NT2 * CK], bf16, tag="kh",
                                      name=f"kh{l}_{hp}")
                        g, dti = (0, hp) if hp < DH2 else (1, hp - DH2)
                        nc.sync.dma_start(
                            kh[:].rearrange("p (r c) -> p r c", c=OT),
                            kout[g][:].rearrange("(r dt p c) -> dt p r c",
                                                 dt=DH2, p=128, c=OT)[dti])
                    vh = vhp.tile([CK, NT2 * 64], bf16, tag="vh",
                                  name=f"vh{l}_{h}")
                    gv, hc = (0, h) if h < H // 2 else (1, h - H // 2)
                    for r in range(NC):
                        nc.sync.dma_start(
                            vh[:].rearrange("p (r q e) -> p r q e",
                                            q=2 * cpb, e=64)[:, r],
                            vout[gv][r * OT * HD2:(r + 1) * OT * HD2]
                            .rearrange("(q p e) -> p q e", p=CK,
                                       e=HD2)[:, :, hc * 64:(hc + 1) * 64])
                    zpv = ps_zv.tile([64, OT], f32, tag="zpv", name=f"zpv{l}_{h}")
                    zps = ps_zs.tile([1, OT], f32, tag="zps", name=f"zps{l}_{h}")
                    for jg in range(NJG):
                        b = 0 if jg < NJGB else 1
                        sc = ps_sc.tile([CK, CGE * QB], f32, tag="sc",
                                        name=f"sc{l}_{h}_{jg}")
                        qh = QT[off:off + 64,
                                hp * OT + b * QB: hp * OT + b * QB + QB]
                        for jj in range(CGE):
                            j = order[jg * CGE + jj]
                            nc.tensor.matmul(
                                sc[:, jj * QB:(jj + 1) * QB],
                                kh[off:off + 64, j * CK:(j + 1) * CK],
                                qh, start=True, stop=True)
                        pu = pup.tile([CK, CGE * QB], bf16, tag="pu",
                                      name=f"pu{l}_{h}_{jg}")
                        nc.scalar.activation(pu[:], sc[:], Exp, scale=0.125)
                        pm = pmp.tile([CK, CGE * QB], bf16, tag="pm",
                                      name=f"pmt{l}_{h}_{jg}")
                        moff = (jg % NJGB) * CGE * QB
                        nc.vector.tensor_mul(
                            pm[:], pu[:], MS[:, moff:moff + CGE * QB])
                        for jj in range(CGE):
                            idx = jg * CGE + jj
                            j = order[idx]
                            st_f = idx in (0, NT2 // 2)
                            sp_f = idx in (NT2 // 2 - 1, NT2 - 1)
                            nc.tensor.matmul(
                                zpv[:, b * QB:(b + 1) * QB],
                                vh[:, j * 64:(j + 1) * 64],
                                pm[:, jj * QB:(jj + 1) * QB],
                                start=st_f, stop=sp_f)
                            nc.tensor.matmul(
                                zps[:, b * QB:(b + 1) * QB],
                                ONE_COL[:CK, :],
                                pu[:, jj * QB:(jj + 1) * QB],
                                start=st_f, stop=sp_f)
                    zs = zsbp.tile([65, OT], f32, tag="zs", name=f"zsb{l}_{h}")
                    nc.scalar.activation(zs[0:64, :], zpv[:], Copy)
                    nc.scalar.activation(zs[64:65, :], zps[:], Copy)
                    zs_list.append(zs)
                    if h % 4 == 3:
                        g4 = h // 4
                        for st in range(ST):
                            tp = ps_sm.tile([128, 260], f32, tag="sm",
                                            name=f"tpz{l}_{g4}_{st}")
                            for gg in range(4):
                                nc.tensor.transpose(
                                    tp[:, gg * 65:(gg + 1) * 65],
                                    zs_list[gg][:, st * 128:(st + 1) * 128],
                                    IDF[0:65, 0:65])
                            rc = smp.tile([128, 4], f32, tag="rc",
                                          name=f"rc{l}_{g4}_{st}")
                            nc.vector.reciprocal(
                                rc[:],
                                tp[:].rearrange("p (g e) -> p g e",
                                                e=65)[:, :, 64])
                            for gg in range(4):
                                hh = g4 * 4 + gg
                                nc.vector.tensor_scalar_mul(
                                    Zacc[:, st * D + hh * 64:
                                         st * D + hh * 64 + 64],
                                    tp[:, gg * 65: gg * 65 + 64],
                                    rc[:, gg:gg + 1])
                        zs_list.clear()

                wf_t = [wp.tile([128, D], bf16, tag="w", name=f"wf{l}_{i}")
                        for i in range(DT)]
                for k in range(DT):
                    nc.sync.dma_start(
                        wf_t[k][:],
                        t_wf.ap()[l].rearrange("(k p) m -> p k m", p=128)[:, k])
                res_ln_cast(f"a{l}", lambda st, xs: nc.vector.tensor_add(
                    xs, xs, Zacc[:, st * D:(st + 1) * D]))

                def ffn_radd(st, xs):
                    fs = sbx.tile([128, D], f32, tag="fs", name=f"fs{l}_{st}")
                    for (n0, nn) in dcols:
                        ps = ps_sm.tile([128, 512], f32, tag="sm",
                                        name=f"fp{l}_{st}_{n0}")
                        for k in range(DT):
                            nc.tensor.matmul(
                                ps[:, :nn],
                                XT[:, k * OT + st * 128: k * OT + (st + 1) * 128],
                                wf_t[k][:, n0:n0 + nn],
                                start=(k == 0), stop=False)
                        nc.tensor.matmul(ps[:, :nn], ONE_ROW[:],
                                         BF[:, l * D + n0: l * D + n0 + nn],
                                         start=False, stop=True)
                        nc.scalar.activation(fs[:, n0:n0 + nn],
                                             ps[:, :nn], Relu)
                    nc.vector.tensor_add(xs, xs, fs[:])

                res_ln_cast(f"f{l}", ffn_radd)

            # ---- final: last-token logits (both batches live on rank NC-1) ----
            xl = sbx.tile([128, 2 * DT], bf16, tag="xl")
            xv = XT[:].rearrange("p (dt c) -> p dt c", c=OT)
            for b in range(2):
                nc.vector.tensor_copy(
                    xl[:].rearrange("p (dt t) -> p dt t", t=2)[:, :, b],
                    xv[:, :, b * QB + QB - 1])
            xlin = dr.tile([128, 2 * DT], bf16, tag="xlin")
            xlout = dr.tile([NC * 128, 2 * DT], bf16, tag="xlout",
                            addr_space="Shared")
            nc.sync.dma_start(xlin[:], xl[:])
            nc.gpsimd.collective_compute(
                "AllGather", mybir.AluOpType.bypass, replica_groups=g8,
                ins=[xlin[:].opt()], outs=[xlout[:].opt()])
            x2 = pp.tile([128, 2 * DT], bf16, tag="x2")
            nc.sync.dma_start(x2[:], xlout[(NC - 1) * 128: NC * 128, :])
            ONE2 = pp.tile([1, 2], bf16, tag="ONE2")
            nc.vector.memset(ONE2[:], 1.0)
            prd = dr.tile([2, VS], f32, tag="prd")  # DRAM staging for exp(logits)
            acc = pp.tile([2, len(vch)], f32, tag="acc")
            for vc, (v0, vn) in enumerate(vch):
                ps = ps_sm.tile([2, 512], f32, tag="sm", name=f"vop{vc}")
                for dt in range(DT):
                    wt = vocab_wt(vc, dt)
                    nc.tensor.matmul(ps[:, :vn], x2[:, 2 * dt:2 * dt + 2],
                                     wt[:, :vn], start=(dt == 0), stop=False)
                bo = smp.tile([1, 512], bf16, tag="bo", name=f"bo{vc}")
                nc.sync.dma_start(bo[:, :vn], t_bout.ap()[:, v0:v0 + vn])
                nc.tensor.matmul(ps[:, :vn], ONE2[:], bo[:, :vn],
                                 start=False, stop=True)
                pre = sbx.tile([2, 512], f32, tag="pre", name=f"pre{vc}")
                nc.scalar.activation(pre[:, :vn], ps[:, :vn], Exp,
                                     accum_out=acc[:, vc:vc + 1])
                nc.sync.dma_start(prd[:, v0:v0 + vn], pre[:, :vn])
            ssum = pp.tile([2, 1], f32, tag="ssum")
            nc.vector.reduce_sum(ssum[:], acc[:], axis=AX)
            esin = dr.tile([2, 1], f32, tag="esin")
            esout = dr.tile([2, 1], f32, tag="esout", addr_space="Shared")
            nc.sync.dma_start(esin[:], ssum[:])
            nc.gpsimd.collective_compute(
                "AllReduce", mybir.AluOpType.add, replica_groups=g8,
                ins=[esin[:].opt()], outs=[esout[:].opt()])
            es = pp.tile([2, 1], f32, tag="es")
            nc.sync.dma_start(es[:], esout[:])
            nc.vector.reciprocal(es[:], es[:])
            for vc, (v0, vn) in enumerate(vch):
                pf = sbx.tile([2, 512], f32, tag="pf", name=f"pf{vc}")
                nc.sync.dma_start(pf[:, :vn], prd[:, v0:v0 + vn])
                nc.vector.tensor_scalar_mul(pf[:, :vn], pf[:, :vn], es[:])
                nc.sync.dma_start(t_probs.ap()[:, v0:v0 + vn], pf[:, :vn])

    nc.compile()
    return nc


def prep_inputs(cfg: Cfg, inputs):
    X = np.asarray(inputs["X"], np.float32)
    Wq = np.asarray(inputs["Wq"], np.float32)
    Wk = np.asarray(inputs["Wk"], np.float32)
    Wv = np.asarray(inputs["Wv"], np.float32)
    bq = np.asarray(inputs["bq"], np.float32)
    bk = np.asarray(inputs["bk"], np.float32)
    bv = np.asarray(inputs["bv"], np.float32)
    Wf = np.asarray(inputs["Wffn"], np.float32)
    bf = np.asarray(inputs["bffn"], np.float32)
    Wo = np.asarray(inputs["Wout"], np.float32)
    bo = np.asarray(inputs["bout"], np.float32)
    L, D, QB, CK, NT2, VS = cfg.L, cfg.D, cfg.QB, cfg.CK, cfg.NT2, cfg.VS

    wq = Wq.transpose(0, 2, 1, 3).reshape(L, D, D).astype(BF16)
    wk = Wk.transpose(0, 2, 1, 3).reshape(L, D, D).astype(BF16)
    wv = Wv.transpose(0, 2, 1, 3).reshape(L, D, D).astype(BF16)
    wf = Wf.astype(BF16)
    bqf = bq.reshape(L, D).astype(np.float32)
    bkf = bk.reshape(L, D).astype(np.float32)
    bvb = bv.reshape(1, L * D).astype(BF16)
    bfb = bf.reshape(1, L * D).astype(BF16)
    idb = np.eye(128, dtype=BF16)
    idf = np.eye(128, dtype=np.float32)
    cpb = QB // CK

    in_maps = []
    for c in range(cfg.NCORES):
        x0 = np.concatenate([X[0, c * QB:(c + 1) * QB],
                             X[1, c * QB:(c + 1) * QB]], axis=0)
        x0 = np.ascontiguousarray(x0)
        x0T = np.ascontiguousarray(x0.T.astype(BF16))
        # chunk j = (rank r, batch b, sub u): key pos t = QB*r + CK*u + p in
        # batch b; query col x of same batch: query pos s = QB*c + x; keep
        # t<=s. Masks are batch-independent, so only the batch-0 half of the
        # chunk order is materialized.
        order0 = [j for j in range(NT2) if (j % (2 * cpb)) // cpb == 0]
        masks = np.zeros((NT2 // 2, CK, QB), np.float32)
        for idx, j in enumerate(order0):
            r = j // (2 * cpb)
            u = j % cpb
            t = QB * r + CK * u + np.arange(CK)[:, None]
            s = QB * c + np.arange(QB)[None, :]
            masks[idx] = (t <= s)
        in_maps.append({
            "x0": x0, "x0T": x0T, "wq": wq, "wk": wk, "wv": wv, "wf": wf,
            "bq": bqf, "bk": bkf, "bv": bvb, "bfn": bfb,
            "masks": np.ascontiguousarray(masks.astype(BF16)),
            "idenb": idb, "idenf": idf,
            "wout": np.ascontiguousarray(Wo[:, c * VS:(c + 1) * VS].astype(BF16)),
            "bout": np.ascontiguousarray(bo[None, c * VS:(c + 1) * VS].astype(BF16)),
        })
    return in_maps


def run(cfg: Cfg, inputs, trace=False):
    from concourse.bass_utils import run_bass_kernel_spmd
    if cfg not in _CACHE:
        _CACHE[cfg] = build_nc(cfg)
    nc = _CACHE[cfg]
    in_maps = prep_inputs(cfg, inputs)
    r = run_bass_kernel_spmd(nc, in_maps, core_ids=list(range(cfg.NCORES)),
                             trace=trace)
    probs = np.concatenate([r.results[c]["probs"] for c in range(cfg.NCORES)],
                           axis=1).astype(np.float32)
    return probs, r


def kernel(**inputs) -> np.ndarray:
    probs, _ = run(FULL, inputs, trace=TRACE)
    return probs



# revision 24
# speedup vs baseline: 1.0221x; 1.0221x over previous
"""Trainium2 Bass kernel for nn_Decoder (4-layer dense transformer decoder).

Sharding: token-parallel over 8 cores, batch-interleaved. Core c owns token
range [QB*c, QB*(c+1)) of BOTH batches (QB = S/8), so the per-layer K/V
AllGathers run over all 8 ranks and every core consumes the gathered buffer
at identical addresses (SPMD-uniform); causality and batch separation are
pure per-core mask data. Attention is computed transposed (S^T[t,s]) so the
full-row softmax denominator (softmax-then-tril, per the reference) falls out
of ones-column matmuls. K/V gathers are split in half (head groups) so the
collective latency pipelines against projection+score compute; masks are
batch-deduplicated; PSUM->SBUF drains run on the scalar engine to keep DVE
off the critical path; vocab weights prefetch during the last layer. Final
vocab projection is last-token-only and vocab-sharded across the 8 cores.

Perf notes from trace analysis (for future iterations): the attention phase
is matmul *instruction-count* bound (~131ns/instruction floor at N=256, PE
back-to-back), not FLOP bound — fp8 DoubleRow QK (tried) halves stream time
but not instruction count and measured slower; zpv/zps DoubleRow chunk-pairs
are ISA-illegal (dual-fp8 LDWEIGHTS caps contraction at 128: 128part x 2
violates s3_lw_dual_fp8_restrictions). Merging zpv+zps into one PSUM bank
(tried twice) serializes PE ~60% — keep them in separate banks. Remaining
headroom: ~45us/layer of exposed collective latency at layer boundaries
and the 32 ones-column denominator matmuls per head. Batch-split pipelining
of the FFN->proj->gather tail (tried, 2177us vs 1928us here) LOST ~250us:
halving projection N to 256 doubles projection instruction count at the
~131ns/instruction floor, and the SBUF cost of the doubled weight pools
forced pu/pm pools down to 4, throttling the attention pipeline. A future
attempt must keep projections at N=512 (pipeline at the gather level only)
and must not shrink pu/pm below 5. LayerNorm now computes var = E[x^2]-mu^2
with Sum(x^2) on the scalar engine (Square+accum_out) and a single fused
normalize pass on DVE (-30us vs the 5-pass DVE version); the FFN ReLU also
runs on ACT. Final trace (1.891ms, PE 70.7% busy): the DVE tail
spikes are fixed (DVE <=49% everywhere; attention buckets run PE 99%/ACT 66%).
Remaining losses, measured: 50us layer-0 cold start (gathers with nothing to
overlap), ~253us of 5-20us gaps in the per-layer collective windows (CC
67-83% while PE dips -- needs gather-level pipelining keeping N=512), ~110us
of sub-us instruction-floor stalls, and the 32 ones-column denominator
matmuls per head inside the PE-saturated attention phase.
"""

import numpy as np
import ml_dtypes
from dataclasses import dataclass

BF16 = ml_dtypes.bfloat16
TRACE = False
_CACHE = {}


@dataclass(frozen=True)
class Cfg:
    L: int = 4
    S: int = 2048       # seq len per batch
    D: int = 1024
    H: int = 16
    V: int = 32000
    DH: int = 64
    NCORES: int = 8
    CGE: int = 4        # key chunks per exp/mask op
    SV_D: int = 2       # score-phase lookahead depth (heads)

    @property
    def QB(self):  # tokens per batch per core
        return self.S // self.NCORES

    @property
    def OT(self):  # tokens per core (both batches)
        return 2 * self.QB

    @property
    def CK(self):  # key-chunk size (contraction tile of attention)
        return min(128, self.QB)

    @property
    def NT2(self):  # total key chunks (both batches)
        return 2 * self.S // self.CK

    @property
    def ST(self):  # 128-row token tiles per core
        return self.OT // 128

    @property
    def DT(self):
        return self.D // 128

    @property
    def VS(self):
        return self.V // self.NCORES


FULL = Cfg()


def _vchunks(vs):
    out, v0 = [], 0
    while v0 < vs:
        vn = min(512, vs - v0)
        out.append((v0, vn))
        v0 += vn
    return out


def build_nc(cfg: Cfg):
    import concourse.bass as bass  # noqa: F401
    from concourse import bacc, mybir, tile

    f32 = mybir.dt.float32
    bf16 = mybir.dt.bfloat16
    f8 = mybir.dt.float8e4
    DR = mybir.MatmulPerfMode.DoubleRow
    L, D, H = cfg.L, cfg.D, cfg.H
    OT, QB, CK, NT2, ST, DT = cfg.OT, cfg.QB, cfg.CK, cfg.NT2, cfg.ST, cfg.DT
    VS, CGE = cfg.VS, cfg.CGE
    NC = cfg.NCORES
    DH2 = DT // 2          # weight m-tiles per gather half
    HD2 = D // 2           # V columns per gather half
    dcols = [(i * 512, min(512, D - i * 512)) for i in range((D + 511) // 512)]
    g8 = [list(range(NC))]
    cpb = QB // CK  # key chunks per rank per batch
    NJG = NT2 // CGE          # jg groups per head (both batches)
    NJGB = NJG // 2           # jg groups per batch

    def bat(j):
        return (j % (2 * cpb)) // cpb

    order = ([j for j in range(NT2) if bat(j) == 0]
             + [j for j in range(NT2) if bat(j) == 1])

    nc = bacc.Bacc("TRN2", target_bir_lowering=False, debug=False,
                   num_devices=NC)

    t_x0 = nc.dram_tensor("x0", [OT, D], f32, kind="ExternalInput")
    t_x0T = nc.dram_tensor("x0T", [D, OT], bf16, kind="ExternalInput")
    t_wq = nc.dram_tensor("wq", [L, D, D], bf16, kind="ExternalInput")
    t_wk = nc.dram_tensor("wk", [L, D, D], bf16, kind="ExternalInput")
    t_wv = nc.dram_tensor("wv", [L, D, D], bf16, kind="ExternalInput")
    t_wf = nc.dram_tensor("wf", [L, D, D], bf16, kind="ExternalInput")
    t_bq = nc.dram_tensor("bq", [L, D], f32, kind="ExternalInput")
    t_bk = nc.dram_tensor("bk", [L, D], f32, kind="ExternalInput")
    t_bv = nc.dram_tensor("bv", [1, L * D], bf16, kind="ExternalInput")
    t_bf = nc.dram_tensor("bfn", [1, L * D], bf16, kind="ExternalInput")
    t_masks = nc.dram_tensor("masks", [NT2 // 2, CK, QB], bf16,
                             kind="ExternalInput")
    t_idb = nc.dram_tensor("idenb", [128, 128], bf16, kind="ExternalInput")
    t_idf = nc.dram_tensor("idenf", [128, 128], f32, kind="ExternalInput")
    t_wout = nc.dram_tensor("wout", [D, VS], bf16, kind="ExternalInput")
    t_bout = nc.dram_tensor("bout", [1, VS], bf16, kind="ExternalInput")
    t_probs = nc.dram_tensor("probs", [2, VS], f32, kind="ExternalOutput")

    Exp = mybir.ActivationFunctionType.Exp
    Copy = mybir.ActivationFunctionType.Copy
    Square = mybir.ActivationFunctionType.Square
    Relu = mybir.ActivationFunctionType.Relu
    AX = mybir.AxisListType.X
    mult = mybir.AluOpType.mult

    with tile.TileContext(nc) as tc:
        with (
            tc.tile_pool(name="persist", bufs=1) as pp,
            tc.tile_pool(name="wpool", bufs=16) as wp,
            tc.tile_pool(name="wout_p", bufs=12) as wop,
            tc.tile_pool(name="khp", bufs=2) as khp,
            tc.tile_pool(name="vhp", bufs=3) as vhp,
            tc.tile_pool(name="pu", bufs=6) as pup,
            tc.tile_pool(name="pm", bufs=6) as pmp,
            tc.tile_pool(name="zsb", bufs=4) as zsbp,
            tc.tile_pool(name="small", bufs=4) as smp,
            tc.tile_pool(name="sbx", bufs=2) as sbx,
            tc.tile_pool(name="ps_sc", bufs=2, space="PSUM") as ps_sc,
            tc.tile_pool(name="ps_zv", bufs=1, space="PSUM") as ps_zv,
            tc.tile_pool(name="ps_zs", bufs=1, space="PSUM") as ps_zs,
            tc.tile_pool(name="ps_sm", bufs=2, space="PSUM") as ps_sm,
            tc.tile_pool(name="dram", bufs=2, space="DRAM") as dr,
        ):
            X = pp.tile([128, ST * D], f32, tag="X")
            Zacc = pp.tile([128, ST * D], bf16, tag="Zacc")
            XT = pp.tile([128, DT * OT], bf16, tag="XT")
            QT = pp.tile([128, DT * OT], bf16, tag="QT")
            MS = pp.tile([CK, (NT2 // 2) * QB], bf16, tag="MS")
            BQ = pp.tile([128, L * DT], f32, tag="BQ")
            BK = pp.tile([128, L * DT], f32, tag="BK")
            BV = pp.tile([1, L * D], bf16, tag="BV")
            BF = pp.tile([1, L * D], bf16, tag="BF")
            IDB = pp.tile([128, 128], bf16, tag="IDB")
            IDF = pp.tile([128, 128], f32, tag="IDF")
            ONE_COL = pp.tile([128, 1], bf16, tag="ONE_COL")
            ONE_ROW = pp.tile([1, 128], bf16, tag="ONE_ROW")

            # XT first, split per k-tile, so the layer-0 K projection's
            # k-loop starts as soon as its first 128-row slice lands.
            x0T_r = t_x0T.ap().rearrange("(dt p) c -> p dt c", p=128)
            for dt in range(DT):
                nc.sync.dma_start(XT[:, dt * OT:(dt + 1) * OT], x0T_r[:, dt])
            nc.sync.dma_start(
                X[:].rearrange("p (st d) -> p st d", d=D),
                t_x0.ap().rearrange("(st p) d -> p st d", p=128))
            nc.sync.dma_start(
                MS[:].rearrange("p (j c) -> p j c", c=QB),
                t_masks.ap().rearrange("j p c -> p j c"))
            nc.sync.dma_start(
                BQ[:].rearrange("p (l dt) -> p l dt", dt=DT),
                t_bq.ap().rearrange("l (dt p) -> p l dt", p=128))
            nc.sync.dma_start(
                BK[:].rearrange("p (l dt) -> p l dt", dt=DT),
                t_bk.ap().rearrange("l (dt p) -> p l dt", p=128))
            nc.sync.dma_start(BV[:], t_bv.ap())
            nc.sync.dma_start(BF[:], t_bf.ap())
            nc.sync.dma_start(IDB[:], t_idb.ap())
            nc.sync.dma_start(IDF[:], t_idf.ap())
            nc.vector.memset(ONE_COL[:], 1.0)
            nc.vector.memset(ONE_ROW[:], 1.0)

            def res_ln_cast(lbl, radd):
                # Residual adds for all token tiles, then the LN chains
                # (pipelined across st on DVE/ACT), then casts, then the
                # transposes dt-major so XT k-tiles complete in k order --
                # the next layer's K projection k-loop starts ~6us sooner.
                # var = E[x^2] - mu^2 (no eps, divides by var per reference).
                # Sum(x^2) rides the scalar engine (Square + accum_out), the
                # mean stays on DVE, normalize is one fused DVE pass:
                # out = x*rv + (-mu*rv), rv = 1/var.
                for st in range(ST):
                    radd(st, X[:, st * D:(st + 1) * D])
                for st in range(ST):
                    xs = X[:, st * D:(st + 1) * D]
                    mu = smp.tile([128, 1], f32, tag="mu", name=f"mu{lbl}_{st}")
                    nc.vector.reduce_sum(mu[:], xs, axis=AX)
                    nc.vector.tensor_scalar_mul(mu[:], mu[:], 1.0 / D)
                    sq = sbx.tile([128, D], bf16, tag="cx", bufs=4,
                                  name=f"sq{lbl}_{st}")
                    vv = smp.tile([128, 1], f32, tag="vv", name=f"vv{lbl}_{st}")
                    nc.scalar.activation(sq[:], xs, Square,
                                         accum_out=vv[:])
                    m2 = smp.tile([128, 1], f32, tag="m2", name=f"m2{lbl}_{st}")
                    nc.vector.tensor_mul(m2[:], mu[:], mu[:])
                    nc.vector.tensor_scalar_mul(vv[:], vv[:], 1.0 / D)
                    nc.vector.tensor_sub(vv[:], vv[:], m2[:])
                    nc.vector.reciprocal(vv[:], vv[:])
                    nmr = smp.tile([128, 1], f32, tag="nmr",
                                   name=f"nmr{lbl}_{st}")
                    nc.vector.tensor_mul(nmr[:], mu[:], vv[:])
                    nc.vector.tensor_scalar_mul(nmr[:], nmr[:], -1.0)
                    nc.vector.tensor_scalar(
                        out=xs, in0=xs, scalar1=vv[:], scalar2=nmr[:],
                        op0=mult, op1=mybir.AluOpType.add)
                cxs = []
                for st in range(ST):
                    cx = sbx.tile([128, D], bf16, tag="cx", bufs=4,
                                  name=f"cx{lbl}_{st}")
                    nc.scalar.activation(cx[:], X[:, st * D:(st + 1) * D], Copy)
                    cxs.append(cx)
                for dt in range(DT):
                    for st in range(ST):
                        tp = ps_sm.tile([128, 128], bf16, tag="sm",
                                        name=f"tpx{lbl}_{st}_{dt}")
                        nc.tensor.transpose(
                            tp[:], cxs[st][:, dt * 128:(dt + 1) * 128], IDB[:])
                        nc.scalar.activation(
                            XT[:, dt * OT + st * 128: dt * OT + (st + 1) * 128],
                            tp[:], Copy)

            vch = _vchunks(VS)
            wt_all = {}

            def vocab_wt(vc, dt):
                v0, vn = vch[vc]
                if (vc, dt) not in wt_all:
                    wt = wop.tile([128, 512], bf16, tag="wo",
                                  name=f"wo{vc}_{dt}")
                    nc.sync.dma_start(wt[:, :vn],
                                      t_wout.ap()[dt * 128:(dt + 1) * 128,
                                                  v0:v0 + vn])
                    wt_all[(vc, dt)] = wt
                return wt_all[(vc, dt)]

            for l in range(L):
                # ---- K projection (feature-major), gathered in 2 halves ----
                kin = [dr.tile([DH2 * 128 * OT], bf16, tag=f"kin{g}",
                               name=f"kin{l}_{g}") for g in range(2)]
                kout = [dr.tile([NC * DH2 * 128 * OT], bf16, tag=f"kout{g}",
                                name=f"kout{l}_{g}", addr_space="Shared")
                        for g in range(2)]
                vin = [dr.tile([OT * HD2], bf16, tag=f"vin{g}",
                               name=f"vin{l}_{g}") for g in range(2)]
                vout = [dr.tile([NC * OT * HD2], bf16, tag=f"vout{g}",
                                name=f"vout{l}_{g}", addr_space="Shared")
                        for g in range(2)]
                # K and V weights live simultaneously (wp bufs=16) so the
                # gather queue can run K0, V0, K1, V1: attention head 0 only
                # blocks on K0+V0, and K1/V1 hide under heads 0-7. Q proj
                # slots between the half-0 and half-1 work to keep PE fed
                # while the half-0 gathers run on CC.
                wk_t = [wp.tile([128, D], bf16, tag="w", name=f"wk{l}_{i}")
                        for i in range(DT)]
                for k in range(DT):
                    nc.sync.dma_start(
                        wk_t[k][:],
                        t_wk.ap()[l].rearrange("(k p) m -> p k m", p=128)[:, k])
                wv_t = [wp.tile([128, D], bf16, tag="w", name=f"wv{l}_{i}")
                        for i in range(DT)]
                for k in range(DT):
                    nc.sync.dma_start(
                        wv_t[k][:],
                        t_wv.ap()[l].rearrange("(k p) m -> p k m", p=128)[:, k])

                def k_half(g):
                    for mi in range(DH2):
                        m = g * DH2 + mi
                        ps = ps_sm.tile([128, OT], f32, tag="sm",
                                        name=f"kp{l}_{m}")
                        for k in range(DT):
                            nc.tensor.matmul(
                                ps[:], wk_t[k][:, m * 128:(m + 1) * 128],
                                XT[:, k * OT:(k + 1) * OT],
                                start=(k == 0), stop=(k == DT - 1))
                        ks = sbx.tile([128, OT], bf16, tag="ks",
                                      name=f"ks{l}_{m}")
                        nc.vector.tensor_scalar_add(
                            ks[:], ps[:], BK[:, l * DT + m: l * DT + m + 1])
                        nc.sync.dma_start(
                            kin[g][mi * 128 * OT:(mi + 1) * 128 * OT]
                            .rearrange("(p c) -> p c", c=OT), ks[:])
                    nc.gpsimd.collective_compute(
                        "AllGather", mybir.AluOpType.bypass, replica_groups=g8,
                        ins=[kin[g][:].opt()], outs=[kout[g][:].opt()])

                def v_half(g):
                    n0, nn = dcols[g]
                    for st in range(ST):
                        ps = ps_sm.tile([128, 512], f32, tag="sm",
                                        name=f"vp{l}_{st}_{n0}")
                        for k in range(DT):
                            nc.tensor.matmul(
                                ps[:, :nn],
                                XT[:, k * OT + st * 128: k * OT + (st + 1) * 128],
                                wv_t[k][:, n0:n0 + nn],
                                start=(k == 0), stop=False)
                        nc.tensor.matmul(ps[:, :nn], ONE_ROW[:],
                                         BV[:, l * D + n0: l * D + n0 + nn],
                                         start=False, stop=True)
                        vs = sbx.tile([128, 512], bf16, tag="vs",
                                      name=f"vsb{l}_{st}_{n0}")
                        nc.vector.tensor_copy(vs[:, :nn], ps[:, :nn])
                        nc.sync.dma_start(
                            vin[g][st * 128 * HD2:(st + 1) * 128 * HD2]
                            .rearrange("(p d) -> p d", d=HD2), vs[:, :nn])
                    nc.gpsimd.collective_compute(
                        "AllGather", mybir.AluOpType.bypass, replica_groups=g8,
                        ins=[vin[g][:].opt()], outs=[vout[g][:].opt()])

                k_half(0)
                v_half(0)
                k_half(1)
                v_half(1)

                # ---- Q projection (feature-major, stays in SBUF) ----
                # Last: wq reuses wk's tag-"w" buffers, legal only once the
                # K half-1 matmuls (wk's last readers) have been issued.
                wq_t = [wp.tile([128, D], bf16, tag="w", name=f"wq{l}_{i}")
                        for i in range(DT)]
                for k in range(DT):
                    nc.sync.dma_start(
                        wq_t[k][:],
                        t_wq.ap()[l].rearrange("(k p) m -> p k m", p=128)[:, k])
                for m in range(DT):
                    ps = ps_sm.tile([128, OT], f32, tag="sm", name=f"qp{l}_{m}")
                    for k in range(DT):
                        nc.tensor.matmul(ps[:], wq_t[k][:, m * 128:(m + 1) * 128],
                                         XT[:, k * OT:(k + 1) * OT],
                                         start=(k == 0), stop=(k == DT - 1))
                    nc.vector.tensor_scalar_add(QT[:, m * OT:(m + 1) * OT], ps[:],
                                                BQ[:, l * DT + m: l * DT + m + 1])

                if l == L - 1:
                    # Prefetch the first 12 vocab weight tiles (= the free
                    # wop bufs — no buf-wait, so the DMA queues never block)
                    # so the vocab tail starts compute immediately.
                    for i in range(12):
                        vocab_wt(i // DT, i % DT)

                # ---- attention (interleaved per-jg, as scheduled by Tile) ----
                zs_list = []
                kh = None
                for h in range(H):
                    hp, off = h // 2, (h % 2) * 64
                    if h % 2 == 0:
                        kh = khp.tile([128, NT2 * CK], bf16, tag="kh",
                                      name=f"kh{l}_{hp}")
                        g, dti = (0, hp) if hp < DH2 else (1, hp - DH2)
                        nc.sync.dma_start(
                            kh[:].rearrange("p (r c) -> p r c", c=OT),
                            kout[g][:].rearrange("(r dt p c) -> dt p r c",
                                                 dt=DH2, p=128, c=OT)[dti])
                    vh = vhp.tile([CK, NT2 * 64], bf16, tag="vh",
                                  name=f"vh{l}_{h}")
                    gv, hc = (0, h) if h < H // 2 else (1, h - H // 2)
                    for r in range(NC):
                        nc.sync.dma_start(
                            vh[:].rearrange("p (r q e) -> p r q e",
                                            q=2 * cpb, e=64)[:, r],
                            vout[gv][r * OT * HD2:(r + 1) * OT * HD2]
                            .rearrange("(q p e) -> p q e", p=CK,
                                       e=HD2)[:, :, hc * 64:(hc + 1) * 64])
                    zpv = ps_zv.tile([64, OT], f32, tag="zpv", name=f"zpv{l}_{h}")
                    zps = ps_zs.tile([1, OT], f32, tag="zps", name=f"zps{l}_{h}")
                    for jg in range(NJG):
                        b = 0 if jg < NJGB else 1
                        sc = ps_sc.tile([CK, CGE * QB], f32, tag="sc",
                                        name=f"sc{l}_{h}_{jg}")
                        qh = QT[off:off + 64,
                                hp * OT + b * QB: hp * OT + b * QB + QB]
                        for jj in range(CGE):
                            j = order[jg * CGE + jj]
                            nc.tensor.matmul(
                                sc[:, jj * QB:(jj + 1) * QB],
                                kh[off:off + 64, j * CK:(j + 1) * CK],
                                qh, start=True, stop=True)
                        pu = pup.tile([CK, CGE * QB], bf16, tag="pu",
                                      name=f"pu{l}_{h}_{jg}")
                        nc.scalar.activation(pu[:], sc[:], Exp, scale=0.125)
                        pm = pmp.tile([CK, CGE * QB], bf16, tag="pm",
                                      name=f"pmt{l}_{h}_{jg}")
                        moff = (jg % NJGB) * CGE * QB
                        nc.vector.tensor_mul(
                            pm[:], pu[:], MS[:, moff:moff + CGE * QB])
                        for jj in range(CGE):
                            idx = jg * CGE + jj
                            j = order[idx]
                            st_f = idx in (0, NT2 // 2)
                            sp_f = idx in (NT2 // 2 - 1, NT2 - 1)
                            nc.tensor.matmul(
                                zpv[:, b * QB:(b + 1) * QB],
                                vh[:, j * 64:(j + 1) * 64],
                                pm[:, jj * QB:(jj + 1) * QB],
                                start=st_f, stop=sp_f)
                            nc.tensor.matmul(
                                zps[:, b * QB:(b + 1) * QB],
                                ONE_COL[:CK, :],
                                pu[:, jj * QB:(jj + 1) * QB],
                                start=st_f, stop=sp_f)
                    zs = zsbp.tile([65, OT], f32, tag="zs", name=f"zsb{l}_{h}")
                    nc.scalar.activation(zs[0:64, :], zpv[:], Copy)
                    nc.scalar.activation(zs[64:65, :], zps[:], Copy)
                    zs_list.append(zs)
                    if h % 4 == 3:
                        g4 = h // 4
                        for st in range(ST):
                            tp = ps_sm.tile([128, 260], f32, tag="sm",
                                            name=f"tpz{l}_{g4}_{st}")
                            for gg in range(4):
                                nc.tensor.transpose(
                                    tp[:, gg * 65:(gg + 1) * 65],
                                    zs_list[gg][:, st * 128:(st + 1) * 128],
                                    IDF[0:65, 0:65])
                            rc = smp.tile([128, 4], f32, tag="rc",
                                          name=f"rc{l}_{g4}_{st}")
                            nc.vector.reciprocal(
                                rc[:],
                                tp[:].rearrange("p (g e) -> p g e",
                                                e=65)[:, :, 64])
                            for gg in range(4):
                                hh = g4 * 4 + gg
                                nc.scalar.activation(
                                    Zacc[:, st * D + hh * 64:
                                         st * D + hh * 64 + 64],
                                    tp[:, gg * 65: gg * 65 + 64],
                                    Copy, scale=rc[:, gg:gg + 1])
                        zs_list.clear()

                wf_t = [wp.tile([128, D], bf16, tag="w", name=f"wf{l}_{i}")
                        for i in range(DT)]
                for k in range(DT):
                    nc.sync.dma_start(
                        wf_t[k][:],
                        t_wf.ap()[l].rearrange("(k p) m -> p k m", p=128)[:, k])
                res_ln_cast(f"a{l}", lambda st, xs: nc.vector.tensor_add(
                    xs, xs, Zacc[:, st * D:(st + 1) * D]))

                def ffn_radd(st, xs):
                    fs = sbx.tile([128, D], f32, tag="fs", name=f"fs{l}_{st}")
                    for (n0, nn) in dcols:
                        ps = ps_sm.tile([128, 512], f32, tag="sm",
                                        name=f"fp{l}_{st}_{n0}")
                        for k in range(DT):
                            nc.tensor.matmul(
                                ps[:, :nn],
                                XT[:, k * OT + st * 128: k * OT + (st + 1) * 128],
                                wf_t[k][:, n0:n0 + nn],
                                start=(k == 0), stop=False)
                        nc.tensor.matmul(ps[:, :nn], ONE_ROW[:],
                                         BF[:, l * D + n0: l * D + n0 + nn],
                                         start=False, stop=True)
                        nc.scalar.activation(fs[:, n0:n0 + nn],
                                             ps[:, :nn], Relu)
                    nc.vector.tensor_add(xs, xs, fs[:])

                res_ln_cast(f"f{l}", ffn_radd)

            # ---- final: last-token logits (both batches live on rank NC-1) ----
            xl = sbx.tile([128, 2 * DT], bf16, tag="xl")
            xv = XT[:].rearrange("p (dt c) -> p dt c", c=OT)
            for b in range(2):
                nc.vector.tensor_copy(
                    xl[:].rearrange("p (dt t) -> p dt t", t=2)[:, :, b],
                    xv[:, :, b * QB + QB - 1])
            xlin = dr.tile([128, 2 * DT], bf16, tag="xlin")
            xlout = dr.tile([NC * 128, 2 * DT], bf16, tag="xlout",
                            addr_space="Shared")
            nc.sync.dma_start(xlin[:], xl[:])
            nc.gpsimd.collective_compute(
                "AllGather", mybir.AluOpType.bypass, replica_groups=g8,
                ins=[xlin[:].opt()], outs=[xlout[:].opt()])
            x2 = pp.tile([128, 2 * DT], bf16, tag="x2")
            nc.sync.dma_start(x2[:], xlout[(NC - 1) * 128: NC * 128, :])
            ONE2 = pp.tile([1, 2], bf16, tag="ONE2")
            nc.vector.memset(ONE2[:], 1.0)
            prd = dr.tile([2, VS], f32, tag="prd")  # DRAM staging for exp(logits)
            acc = pp.tile([2, len(vch)], f32, tag="acc")
            for vc, (v0, vn) in enumerate(vch):
                ps = ps_sm.tile([2, 512], f32, tag="sm", name=f"vop{vc}")
                for dt in range(DT):
                    wt = vocab_wt(vc, dt)
                    nc.tensor.matmul(ps[:, :vn], x2[:, 2 * dt:2 * dt + 2],
                                     wt[:, :vn], start=(dt == 0), stop=False)
                bo = smp.tile([1, 512], bf16, tag="bo", name=f"bo{vc}")
                nc.sync.dma_start(bo[:, :vn], t_bout.ap()[:, v0:v0 + vn])
                nc.tensor.matmul(ps[:, :vn], ONE2[:], bo[:, :vn],
                                 start=False, stop=True)
                pre = sbx.tile([2, 512], f32, tag="pre", name=f"pre{vc}")
                nc.scalar.activation(pre[:, :vn], ps[:, :vn], Exp,
                                     accum_out=acc[:, vc:vc + 1])
                nc.sync.dma_start(prd[:, v0:v0 + vn], pre[:, :vn])
            ssum = pp.tile([2, 1], f32, tag="ssum")
            nc.vector.reduce_sum(ssum[:], acc[:], axis=AX)
            esin = dr.tile([2, 1], f32, tag="esin")
            esout = dr.tile([2, 1], f32, tag="esout", addr_space="Shared")
            nc.sync.dma_start(esin[:], ssum[:])
            nc.gpsimd.collective_compute(
                "AllReduce", mybir.AluOpType.add, replica_groups=g8,
                ins=[esin[:].opt()], outs=[esout[:].opt()])
            es = pp.tile([2, 1], f32, tag="es")
            nc.sync.dma_start(es[:], esout[:])
            nc.vector.reciprocal(es[:], es[:])
            for vc, (v0, vn) in enumerate(vch):
                pf = sbx.tile([2, 512], f32, tag="pf", name=f"pf{vc}")
                nc.sync.dma_start(pf[:, :vn], prd[:, v0:v0 + vn])
                nc.vector.tensor_scalar_mul(pf[:, :vn], pf[:, :vn], es[:])
                nc.sync.dma_start(t_probs.ap()[:, v0:v0 + vn], pf[:, :vn])

    nc.compile()
    return nc


def prep_inputs(cfg: Cfg, inputs):
    X = np.asarray(inputs["X"], np.float32)
    Wq = np.asarray(inputs["Wq"], np.float32)
    Wk = np.asarray(inputs["Wk"], np.float32)
    Wv = np.asarray(inputs["Wv"], np.float32)
    bq = np.asarray(inputs["bq"], np.float32)
    bk = np.asarray(inputs["bk"], np.float32)
    bv = np.asarray(inputs["bv"], np.float32)
    Wf = np.asarray(inputs["Wffn"], np.float32)
    bf = np.asarray(inputs["bffn"], np.float32)
    Wo = np.asarray(inputs["Wout"], np.float32)
    bo = np.asarray(inputs["bout"], np.float32)
    L, D, QB, CK, NT2, VS = cfg.L, cfg.D, cfg.QB, cfg.CK, cfg.NT2, cfg.VS

    wq = Wq.transpose(0, 2, 1, 3).reshape(L, D, D).astype(BF16)
    wk = Wk.transpose(0, 2, 1, 3).reshape(L, D, D).astype(BF16)
    wv = Wv.transpose(0, 2, 1, 3).reshape(L, D, D).astype(BF16)
    wf = Wf.astype(BF16)
    bqf = bq.reshape(L, D).astype(np.float32)
    bkf = bk.reshape(L, D).astype(np.float32)
    bvb = bv.reshape(1, L * D).astype(BF16)
    bfb = bf.reshape(1, L * D).astype(BF16)
    idb = np.eye(128, dtype=BF16)
    idf = np.eye(128, dtype=np.float32)
    cpb = QB // CK

    in_maps = []
    for c in range(cfg.NCORES):
        x0 = np.concatenate([X[0, c * QB:(c + 1) * QB],
                             X[1, c * QB:(c + 1) * QB]], axis=0)
        x0 = np.ascontiguousarray(x0)
        x0T = np.ascontiguousarray(x0.T.astype(BF16))
        # chunk j = (rank r, batch b, sub u): key pos t = QB*r + CK*u + p in
        # batch b; query col x of same batch: query pos s = QB*c + x; keep
        # t<=s. Masks are batch-independent, so only the batch-0 half of the
        # chunk order is materialized.
        order0 = [j for j in range(NT2) if (j % (2 * cpb)) // cpb == 0]
        masks = np.zeros((NT2 // 2, CK, QB), np.float32)
        for idx, j in enumerate(order0):
            r = j // (2 * cpb)
            u = j % cpb
            t = QB * r + CK * u + np.arange(CK)[:, None]
            s = QB * c + np.arange(QB)[None, :]
            masks[idx] = (t <= s)
        in_maps.append({
            "x0": x0, "x0T": x0T, "wq": wq, "wk": wk, "wv": wv, "wf": wf,
            "bq": bqf, "bk": bkf, "bv": bvb, "bfn": bfb,
            "masks": np.ascontiguousarray(masks.astype(BF16)),
            "idenb": idb, "idenf": idf,
            "wout": np.ascontiguousarray(Wo[:, c * VS:(c + 1) * VS].astype(BF16)),
            "bout": np.ascontiguousarray(bo[None, c * VS:(c + 1) * VS].astype(BF16)),
        })
    return in_maps


def run(cfg: Cfg, inputs, trace=False):
    from concourse.bass_utils import run_bass_kernel_spmd
    if cfg not in _CACHE:
        _CACHE[cfg] = build_nc(cfg)
    nc = _CACHE[cfg]
    in_maps = prep_inputs(cfg, inputs)
    r = run_bass_kernel_spmd(nc, in_maps, core_ids=list(range(cfg.NCORES)),
                             trace=trace)
    probs = np.concatenate([r.results[c]["probs"] for c in range(cfg.NCORES)],
                           axis=1).astype(np.float32)
    return probs, r


def kernel(**inputs) -> np.ndarray:
    probs, _ = run(FULL, inputs, trace=TRACE)
    return probs

